# revision 14
# baseline (speedup 1.0000x reference)
"""Trainium2 Bass kernel: sigmoid(rowdot(tanh(x1@W.T+b), tanh(x2@W.T+b))).

Sharding: pure data-parallel over batch across 8 NeuronCores (B=65536
total -> 8192 rows/core, D_IN=1024, D_PROJ=128).

The kernel is DMA-bound on the activation loads, so the host pre-packs
x1/x2 into fp16 (end-to-end max rel err ~5e-3 vs the 2e-2 gate, measured
on the reference distribution) and into the exact PE-ready transposed
tile layout, halving HBM traffic to 32 MiB/core (~86 us at the measured
~394 GB/s per-NC DMA rate, which is the 16-SDMA-engine limit) and
eliminating every on-device PE transpose:

  xc[t][p, s*4096 + k*BT + b] = xs[t*BT + b, k*128 + p]   (s=0: x1, 1: x2)

Each 512-row tile is loaded by TWO 1 MiB contiguous DMAs (x1 half, x2
half) so the matmuls on x1 start as soon as that half's completion
semaphore fires — one fused DMA per tile would leave PE a full tile
(~5 us) behind the stream, which materializes as drain at the end. The
last tile's x2 half is further split k0-5/k6/k7 so the post-last-byte
critical path is one N=512 matmul + tanh/mul/reduce/sigmoid + one 2 KiB
store (~5 us total drain).

Per tile: 8 fp16 matmuls (N=512, 1 cyc/row warm at 2.4 GHz) accumulate
oT=W.T@xT chunkwise into one PSUM bank; ACT fuses tanh(po+bias)
PSUM->SBUF; same for x2; DVE multiplies; PE reduces partitions via
ones[128,128] matmul (f32r); ACT sigmoid; 2 KiB store from a rotating
partition. PE load is ~4.2 us/tile vs ~5.2 us/tile of DMA, so only the
partition reduce needs manual deferral (emitted between the next tile's
two matmul groups) to avoid an in-order PE stall behind the tanh->mul
chain. wt/bias/ones ride the scalar-engine DMA ring so the sync ring is
purely the x stream; output stores also use the scalar ring.
"""

import numpy as np

import concourse.bacc as bacc
import concourse.mybir as mybir
import concourse.tile as tile
from concourse.bass_utils import run_bass_kernel_spmd

N_CORES = 8
B_TOTAL = 65536
BSH = B_TOTAL // N_CORES  # 8192 rows per core
D_IN = 1024
D_PROJ = 128
P = 128
BT = 512                 # batch tile (matmul moving dim)
NBT = BSH // BT          # 16 batch tiles per core
KC = D_IN // P           # 8 contraction chunks
FW = KC * BT             # 4096 free-dim elements per packed half-tile

F32 = mybir.dt.float32
F32R = mybir.dt.float32r
F16 = mybir.dt.float16


def _build_module():
    nc = bacc.Bacc("TRN2", target_bir_lowering=False, debug=False)

    xc = nc.dram_tensor("xc", [NBT, P, 2 * FW], F16, kind="ExternalInput").ap()
    wt = nc.dram_tensor("wt", [P, KC, D_PROJ], F16, kind="ExternalInput").ap()
    bias = nc.dram_tensor("bias", [P, 1], F32, kind="ExternalInput").ap()
    ones = nc.dram_tensor("ones", [P, P], F32R, kind="ExternalInput").ap()
    out = nc.dram_tensor("out", [BSH], F32, kind="ExternalOutput").ap()

    with tile.TileContext(nc) as tc:
        with (
            tc.tile_pool(name="consts", bufs=1) as cpool,
            tc.tile_pool(name="x", bufs=4) as xpool,
            tc.tile_pool(name="acts", bufs=2) as apool,
            tc.tile_pool(name="po", bufs=6, space="PSUM") as opool,
        ):
            wt_sb = cpool.tile([P, KC, D_PROJ], F16, tag="wt")
            bias_sb = cpool.tile([P, 1], F32, tag="bias")
            ones_sb = cpool.tile([P, P], F32R, tag="ones")

            pending = []

            def flush_pending():
                while pending:
                    prod_p, row0_p, idx_p = pending.pop(0)
                    psim = opool.tile([P, BT], F32, name="psim", tag="po")
                    nc.tensor.matmul(
                        psim,
                        ones_sb,
                        prod_p,
                        start=True,
                        stop=True,
                        skip_group_check=True,
                    )
                    sig = apool.tile([P, BT], F32, tag="sig")
                    nc.scalar.activation(
                        sig, psim, mybir.ActivationFunctionType.Sigmoid
                    )
                    row = (idx_p * 4) % P  # rotate partition -> spread DMA engines
                    nc.scalar.dma_start(
                        out=out[row0_p:row0_p + BT].rearrange(
                            "(a n) -> a n", a=1
                        ),
                        in_=sig[row:row + 1, :],
                    )

            def mm_group(sb, tens, mid=None):
                base = tens * FW
                po = opool.tile([P, BT], F32, name=f"po{tens}", tag="po")
                for k in range(KC):
                    nc.tensor.matmul(
                        po,
                        wt_sb[:, k, :],
                        sb[:, base + k * BT:base + (k + 1) * BT],
                        start=(k == 0),
                        stop=(k == KC - 1),
                        skip_group_check=True,
                    )
                    if k == 2 and mid is not None:
                        mid()
                t_sb = apool.tile([P, BT], F32, tag=f"t{tens}")
                nc.scalar.activation(
                    t_sb, po, mybir.ActivationFunctionType.Tanh, bias=bias_sb
                )
                return t_sb

            loaded = {}

            def load(t):
                sb = xpool.tile([P, 2 * FW], F16, tag="sb")
                nc.sync.dma_start(out=sb[:, :FW], in_=xc[t][:, :FW])
                if t < NBT - 1:
                    # x2 as two k-quarters so the second matmul group
                    # starts mid-window instead of after the full half —
                    # halves the structural PE phase lag vs the stream.
                    cuts = [0, 4 * BT, 8 * BT]
                else:
                    # Final tile: finer x2 split so the post-last-byte
                    # chain is one N=512 matmul + pipelined tail.
                    cuts = [0, 4 * BT, 6 * BT, 7 * BT, 8 * BT]
                for a, b in zip(cuts[:-1], cuts[1:]):
                    nc.sync.dma_start(
                        out=sb[:, FW + a:FW + b],
                        in_=xc[t][:, FW + a:FW + b],
                    )
                loaded[t] = sb

            def compute(t):
                sb = loaded.pop(t)
                # pending reduce of the previous tile rides between the
                # two matmul groups so PE never waits on tanh->mul.
                t1 = mm_group(sb, 0, mid=flush_pending)
                if t < NBT - 1:
                    t2 = mm_group(sb, 1)
                    prod = apool.tile([P, BT], F32R, tag="prod")
                    nc.vector.tensor_mul(prod, t1, t2)
                    pending.append((prod, t * BT, t))
                    return
                # Final tile: run tanh/mul/reduce/sigmoid/store in two
                # 256-column half-chains so the engines pipeline instead
                # of serializing one full-width chain after the last
                # matmul (ACT does half B's tanh while DVE multiplies
                # half A, etc).
                base = FW
                po = opool.tile([P, BT], F32, name="po1", tag="po")
                for k in range(KC):
                    nc.tensor.matmul(
                        po,
                        wt_sb[:, k, :],
                        sb[:, base + k * BT:base + (k + 1) * BT],
                        start=(k == 0),
                        stop=(k == KC - 1),
                        skip_group_check=True,
                    )
                H = BT // 2
                for h in range(2):
                    cs = slice(h * H, (h + 1) * H)
                    t2h = apool.tile([P, H], F32, tag="t1")
                    nc.scalar.activation(
                        t2h, po[:, cs], mybir.ActivationFunctionType.Tanh,
                        bias=bias_sb,
                    )
                    prod = apool.tile([P, H], F32R, tag="prod")
                    nc.vector.tensor_mul(prod, t1[:, cs], t2h)
                    psim = opool.tile([P, H], F32, name="psim", tag="po")
                    nc.tensor.matmul(
                        psim, ones_sb, prod,
                        start=True, stop=True, skip_group_check=True,
                    )
                    sig = apool.tile([P, H], F32, tag="sig")
                    nc.scalar.activation(
                        sig, psim, mybir.ActivationFunctionType.Sigmoid
                    )
                    row = ((2 * t + h) * 4) % P
                    nc.scalar.dma_start(
                        out=out[t * BT + h * H:t * BT + (h + 1) * H]
                        .rearrange("(a n) -> a n", a=1),
                        in_=sig[row:row + 1, :],
                    )

            # Issue order: x tile 0 first on the sync ring (it IS the
            # stream bottleneck); wt/bias/ones ride the otherwise-idle
            # scalar ring (wt gates the first matmul, bias the first
            # tanh, ones the first reduce).
            load(0)
            nc.scalar.dma_start(out=wt_sb, in_=wt)
            nc.scalar.dma_start(out=bias_sb, in_=bias)
            nc.scalar.dma_start(out=ones_sb, in_=ones)
            for t in range(1, NBT):
                load(t)
                compute(t - 1)
            compute(NBT - 1)
            flush_pending()

    nc.compile()
    return nc


_NC_CACHE = None


def _get_module():
    global _NC_CACHE
    if _NC_CACHE is None:
        _NC_CACHE = _build_module()
    return _NC_CACHE


def _pack_x(x):
    """[B, D_IN] fp32 -> [N_CORES, NBT, P, FW] fp16 PE-ready tiles.

    Slot t holds tile t's transposed layout [p, k*BT + b].
    """
    xh = np.asarray(x, dtype=np.float32).astype(np.float16)
    a = xh.reshape(N_CORES, NBT, BT, KC, P).transpose(0, 1, 4, 3, 2)
    return np.ascontiguousarray(a).reshape(N_CORES, NBT, P, FW)


def _pack_inputs(x1, x2, W, b):
    f1 = _pack_x(x1)
    f2 = _pack_x(x2)
    xc_all = np.concatenate([f1, f2], axis=3)  # [c, t, p, 2*FW]
    wt = np.ascontiguousarray(
        np.asarray(W, dtype=np.float32).T.reshape(KC, P, D_PROJ)
        .transpose(1, 0, 2)
    ).astype(np.float16)
    bias = np.ascontiguousarray(np.asarray(b, dtype=np.float32).reshape(P, 1))
    ones = np.ones((P, P), dtype=np.float32)
    return [
        {
            "xc": np.ascontiguousarray(xc_all[i]),
            "wt": wt,
            "bias": bias,
            "ones": ones,
        }
        for i in range(N_CORES)
    ]


def kernel(x1, x2, W, b):
    nc = _get_module()
    in_maps = _pack_inputs(x1, x2, W, b)
    res = run_bass_kernel_spmd(nc, in_maps, core_ids=list(range(N_CORES)))
    return np.concatenate([res.results[i]["out"] for i in range(N_CORES)])


# revision 15
# speedup vs baseline: 1.1456x; 1.1456x over previous
"""Trainium2 Bass kernel: sigmoid(rowdot(tanh(x1@W.T+b), tanh(x2@W.T+b))).

Sharding: pure data-parallel over batch across 8 NeuronCores (B=65536
total -> 8192 rows/core, D_IN=1024, D_PROJ=128).

The kernel is DMA-bound on the activation loads, so the host pre-packs
x1/x2 into fp16 (end-to-end max rel err ~5e-3 vs the 2e-2 gate, measured
on the reference distribution) and into the exact PE-ready transposed
tile layout, halving HBM traffic to 32 MiB/core (~86 us at the measured
~394 GB/s per-NC DMA rate, which is the 16-SDMA-engine limit) and
eliminating every on-device PE transpose:

  xc[t][p, s*4096 + k*BT + b] = xs[t*BT + b, k*128 + p]   (s=0: x1, 1: x2)

Each 512-row tile is loaded by TWO 1 MiB contiguous DMAs (x1 half, x2
half) so the matmuls on x1 start as soon as that half's completion
semaphore fires — one fused DMA per tile would leave PE a full tile
(~5 us) behind the stream, which materializes as drain at the end. The
last tile's x2 half is further split k0-5/k6/k7 so the post-last-byte
critical path is one N=512 matmul + tanh/mul/reduce/sigmoid + one 2 KiB
store (~5 us total drain).

Per tile: 8 fp16 matmuls (N=512, 1 cyc/row warm at 2.4 GHz) accumulate
oT=W.T@xT chunkwise into one PSUM bank; ACT fuses tanh(po+bias)
PSUM->SBUF; same for x2; DVE multiplies; PE reduces partitions via
ones[128,128] matmul (f32r); ACT sigmoid; 2 KiB store from a rotating
partition. PE load is ~4.2 us/tile vs ~5.2 us/tile of DMA, so only the
partition reduce needs manual deferral (emitted between the next tile's
two matmul groups) to avoid an in-order PE stall behind the tanh->mul
chain. wt/bias/ones ride the scalar-engine DMA ring so the sync ring is
purely the x stream; output stores also use the scalar ring.
"""

import numpy as np

import concourse.bacc as bacc
import concourse.mybir as mybir
import concourse.tile as tile
from concourse.bass_utils import run_bass_kernel_spmd

N_CORES = 8
B_TOTAL = 65536
BSH = B_TOTAL // N_CORES  # 8192 rows per core
D_IN = 1024
D_PROJ = 128
P = 128
BT = 512                 # batch tile (matmul moving dim)
NBT = BSH // BT          # 16 batch tiles per core
KC = D_IN // P           # 8 contraction chunks
FW = KC * BT             # 4096 free-dim elements per packed half-tile

F32 = mybir.dt.float32
F32R = mybir.dt.float32r
F16 = mybir.dt.float16


def _build_module():
    nc = bacc.Bacc("TRN2", target_bir_lowering=False, debug=False)

    xc = nc.dram_tensor("xc", [NBT, P, 2 * FW], F16, kind="ExternalInput").ap()
    wt = nc.dram_tensor("wt", [P, KC, D_PROJ], F16, kind="ExternalInput").ap()
    bias = nc.dram_tensor("bias", [P, 1], F32, kind="ExternalInput").ap()
    ones = nc.dram_tensor("ones", [P, P], F16, kind="ExternalInput").ap()
    out = nc.dram_tensor("out", [BSH], F32, kind="ExternalOutput").ap()

    with tile.TileContext(nc) as tc:
        with (
            tc.tile_pool(name="consts", bufs=1) as cpool,
            tc.tile_pool(name="x", bufs=4) as xpool,
            tc.tile_pool(name="acts", bufs=2) as apool,
            tc.tile_pool(name="po", bufs=6, space="PSUM") as opool,
        ):
            wt_sb = cpool.tile([P, KC, D_PROJ], F16, tag="wt")
            bias_sb = cpool.tile([P, 1], F32, tag="bias")
            ones_sb = cpool.tile([P, P], F16, tag="ones")

            pending = []

            def flush_pending():
                while pending:
                    prod_p, row0_p, idx_p = pending.pop(0)
                    psim = opool.tile([P, BT], F32, name="psim", tag="po")
                    nc.tensor.matmul(
                        psim,
                        ones_sb,
                        prod_p,
                        start=True,
                        stop=True,
                        skip_group_check=True,
                    )
                    sig = apool.tile([P, BT], F32, tag="sig")
                    nc.scalar.activation(
                        sig, psim, mybir.ActivationFunctionType.Sigmoid
                    )
                    row = (idx_p * 4) % P  # rotate partition -> spread DMA engines
                    nc.scalar.dma_start(
                        out=out[row0_p:row0_p + BT].rearrange(
                            "(a n) -> a n", a=1
                        ),
                        in_=sig[row:row + 1, :],
                    )

            def mm_group(sb, tens, mid=None):
                base = tens * FW
                po = opool.tile([P, BT], F32, name=f"po{tens}", tag="po")
                for k in range(KC):
                    nc.tensor.matmul(
                        po,
                        wt_sb[:, k, :],
                        sb[:, base + k * BT:base + (k + 1) * BT],
                        start=(k == 0),
                        stop=(k == KC - 1),
                        skip_group_check=True,
                    )
                    if k == 2 and mid is not None:
                        mid()
                t_sb = apool.tile([P, BT], F16, tag=f"t{tens}")
                nc.scalar.activation(
                    t_sb, po, mybir.ActivationFunctionType.Tanh, bias=bias_sb
                )
                return t_sb

            loaded = {}

            def load(t):
                sb = xpool.tile([P, 2 * FW], F16, tag="sb")
                nc.sync.dma_start(out=sb[:, :FW], in_=xc[t][:, :FW])
                if t < NBT - 1:
                    # x2 as two k-quarters so the second matmul group
                    # starts mid-window instead of after the full half —
                    # halves the structural PE phase lag vs the stream.
                    cuts = [0, 4 * BT, 8 * BT]
                else:
                    # Final tile: finer x2 split so the post-last-byte
                    # chain is one N=512 matmul + pipelined tail.
                    cuts = [0, 4 * BT, 6 * BT, 7 * BT, 8 * BT]
                for a, b in zip(cuts[:-1], cuts[1:]):
                    nc.sync.dma_start(
                        out=sb[:, FW + a:FW + b],
                        in_=xc[t][:, FW + a:FW + b],
                    )
                loaded[t] = sb

            def compute(t):
                sb = loaded.pop(t)
                # pending reduce of the previous tile rides between the
                # two matmul groups so PE never waits on tanh->mul.
                t1 = mm_group(sb, 0, mid=flush_pending)
                if t < NBT - 1:
                    t2 = mm_group(sb, 1)
                    prod = apool.tile([P, BT], F16, tag="prod")
                    nc.vector.tensor_mul(prod, t1, t2)
                    pending.append((prod, t * BT, t))
                    return
                # Final tile: run tanh/mul/reduce/sigmoid/store in two
                # 256-column half-chains so the engines pipeline instead
                # of serializing one full-width chain after the last
                # matmul (ACT does half B's tanh while DVE multiplies
                # half A, etc).
                base = FW
                po = opool.tile([P, BT], F32, name="po1", tag="po")
                for k in range(KC):
                    nc.tensor.matmul(
                        po,
                        wt_sb[:, k, :],
                        sb[:, base + k * BT:base + (k + 1) * BT],
                        start=(k == 0),
                        stop=(k == KC - 1),
                        skip_group_check=True,
                    )
                H = BT // 2
                for h in range(2):
                    cs = slice(h * H, (h + 1) * H)
                    t2h = apool.tile([P, H], F16, tag="t1")
                    nc.scalar.activation(
                        t2h, po[:, cs], mybir.ActivationFunctionType.Tanh,
                        bias=bias_sb,
                    )
                    prod = apool.tile([P, H], F16, tag="prod")
                    nc.vector.tensor_mul(prod, t1[:, cs], t2h)
                    psim = opool.tile([P, H], F32, name="psim", tag="po")
                    nc.tensor.matmul(
                        psim, ones_sb, prod,
                        start=True, stop=True, skip_group_check=True,
                    )
                    sig = apool.tile([P, H], F32, tag="sig")
                    nc.scalar.activation(
                        sig, psim, mybir.ActivationFunctionType.Sigmoid
                    )
                    row = ((2 * t + h) * 4) % P
                    nc.sync.dma_start(
                        out=out[t * BT + h * H:t * BT + (h + 1) * H]
                        .rearrange("(a n) -> a n", a=1),
                        in_=sig[row:row + 1, :],
                    )

            # Issue order: x tile 0 first on the sync ring (it IS the
            # stream bottleneck); wt/bias/ones ride the otherwise-idle
            # scalar ring (wt gates the first matmul, bias the first
            # tanh, ones the first reduce).
            load(0)
            nc.scalar.dma_start(out=wt_sb, in_=wt)
            nc.scalar.dma_start(out=bias_sb, in_=bias)
            nc.scalar.dma_start(out=ones_sb, in_=ones)
            for t in range(1, NBT):
                load(t)
                compute(t - 1)
            compute(NBT - 1)
            flush_pending()

    nc.compile()
    return nc


_NC_CACHE = None


def _get_module():
    global _NC_CACHE
    if _NC_CACHE is None:
        _NC_CACHE = _build_module()
    return _NC_CACHE


def _pack_x(x):
    """[B, D_IN] fp32 -> [N_CORES, NBT, P, FW] fp16 PE-ready tiles.

    Slot t holds tile t's transposed layout [p, k*BT + b].
    """
    xh = np.asarray(x, dtype=np.float32).astype(np.float16)
    a = xh.reshape(N_CORES, NBT, BT, KC, P).transpose(0, 1, 4, 3, 2)
    return np.ascontiguousarray(a).reshape(N_CORES, NBT, P, FW)


def _pack_inputs(x1, x2, W, b):
    f1 = _pack_x(x1)
    f2 = _pack_x(x2)
    xc_all = np.concatenate([f1, f2], axis=3)  # [c, t, p, 2*FW]
    wt = np.ascontiguousarray(
        np.asarray(W, dtype=np.float32).T.reshape(KC, P, D_PROJ)
        .transpose(1, 0, 2)
    ).astype(np.float16)
    bias = np.ascontiguousarray(np.asarray(b, dtype=np.float32).reshape(P, 1))
    ones = np.ones((P, P), dtype=np.float16)
    return [
        {
            "xc": np.ascontiguousarray(xc_all[i]),
            "wt": wt,
            "bias": bias,
            "ones": ones,
        }
        for i in range(N_CORES)
    ]


def kernel(x1, x2, W, b):
    nc = _get_module()
    in_maps = _pack_inputs(x1, x2, W, b)
    res = run_bass_kernel_spmd(nc, in_maps, core_ids=list(range(N_CORES)))
    return np.concatenate([res.results[i]["out"] for i in range(N_CORES)])


# revision 16
# speedup vs baseline: 1.1709x; 1.0221x over previous
"""Trainium2 Bass kernel: sigmoid(rowdot(tanh(x1@W.T+b), tanh(x2@W.T+b))).

Sharding: pure data-parallel over batch across 8 NeuronCores (B=65536
total -> 8192 rows/core, D_IN=1024, D_PROJ=128).

The kernel is DMA-bound on the activation loads, so the host pre-packs
x1/x2 into fp16 (end-to-end max rel err ~5e-3 vs the 2e-2 gate, measured
on the reference distribution) and into the exact PE-ready transposed
tile layout, halving HBM traffic to 32 MiB/core (~86 us at the measured
~394 GB/s per-NC DMA rate, which is the 16-SDMA-engine limit) and
eliminating every on-device PE transpose:

  xc[t][p, s*4096 + k*BT + b] = xs[t*BT + b, k*128 + p]   (s=0: x1, 1: x2)

Each 512-row tile is loaded by TWO 1 MiB contiguous DMAs (x1 half, x2
half) so the matmuls on x1 start as soon as that half's completion
semaphore fires — one fused DMA per tile would leave PE a full tile
(~5 us) behind the stream, which materializes as drain at the end. The
last tile's x2 half is further split k0-5/k6/k7 so the post-last-byte
critical path is one N=512 matmul + tanh/mul/reduce/sigmoid + one 2 KiB
store (~5 us total drain).

Per tile: 8 fp16 matmuls (N=512, 1 cyc/row warm at 2.4 GHz) accumulate
oT=W.T@xT chunkwise into one PSUM bank; ACT fuses tanh(po+bias)
PSUM->SBUF; same for x2; DVE multiplies; PE reduces partitions via
ones[128,128] matmul (f32r); ACT sigmoid; 2 KiB store from a rotating
partition. PE load is ~4.2 us/tile vs ~5.2 us/tile of DMA, so only the
partition reduce needs manual deferral (emitted between the next tile's
two matmul groups) to avoid an in-order PE stall behind the tanh->mul
chain. wt/bias/ones ride the scalar-engine DMA ring so the sync ring is
purely the x stream; output stores also use the scalar ring.
"""

import numpy as np

import concourse.bacc as bacc
import concourse.mybir as mybir
import concourse.tile as tile
from concourse.bass_utils import run_bass_kernel_spmd

N_CORES = 8
B_TOTAL = 65536
BSH = B_TOTAL // N_CORES  # 8192 rows per core
D_IN = 1024
D_PROJ = 128
P = 128
BT = 512                 # batch tile (matmul moving dim)
NBT = BSH // BT          # 16 batch tiles per core
KC = D_IN // P           # 8 contraction chunks
FW = KC * BT             # 4096 free-dim elements per packed half-tile

F32 = mybir.dt.float32
F32R = mybir.dt.float32r
F16 = mybir.dt.float16


def _build_module():
    nc = bacc.Bacc("TRN2", target_bir_lowering=False, debug=False)

    xc = nc.dram_tensor("xc", [NBT, P, 2 * FW], F16, kind="ExternalInput").ap()
    wt = nc.dram_tensor("wt", [P, KC, D_PROJ], F16, kind="ExternalInput").ap()
    bias = nc.dram_tensor("bias", [P, 1], F32, kind="ExternalInput").ap()
    ones = nc.dram_tensor("ones", [P, P], F16, kind="ExternalInput").ap()
    out = nc.dram_tensor("out", [BSH], F32, kind="ExternalOutput").ap()

    with tile.TileContext(nc) as tc:
        with (
            tc.tile_pool(name="consts", bufs=1) as cpool,
            tc.tile_pool(name="x", bufs=4) as xpool,
            tc.tile_pool(name="acts", bufs=2) as apool,
            tc.tile_pool(name="po", bufs=6, space="PSUM") as opool,
        ):
            wt_sb = cpool.tile([P, KC, D_PROJ], F16, tag="wt")
            bias_sb = cpool.tile([P, 1], F32, tag="bias")
            ones_sb = cpool.tile([P, P], F16, tag="ones")

            pending = []

            def flush_pending():
                while pending:
                    prod_p, row0_p, idx_p = pending.pop(0)
                    psim = opool.tile([P, BT], F32, name="psim", tag="po")
                    nc.tensor.matmul(
                        psim,
                        ones_sb,
                        prod_p,
                        start=True,
                        stop=True,
                        skip_group_check=True,
                    )
                    sig = apool.tile([P, BT], F32, tag="sig")
                    nc.scalar.activation(
                        sig, psim, mybir.ActivationFunctionType.Sigmoid
                    )
                    row = (idx_p * 4) % P  # rotate partition -> spread DMA engines
                    nc.scalar.dma_start(
                        out=out[row0_p:row0_p + BT].rearrange(
                            "(a n) -> a n", a=1
                        ),
                        in_=sig[row:row + 1, :],
                    )

            def mm_group(sb, tens, mid=None):
                base = tens * FW
                po = opool.tile([P, BT], F32, name=f"po{tens}", tag="po")
                for k in range(KC):
                    nc.tensor.matmul(
                        po,
                        wt_sb[:, k, :],
                        sb[:, base + k * BT:base + (k + 1) * BT],
                        start=(k == 0),
                        stop=(k == KC - 1),
                        skip_group_check=True,
                    )
                    if k == 2 and mid is not None:
                        mid()
                t_sb = apool.tile([P, BT], F16, tag=f"t{tens}")
                nc.scalar.activation(
                    t_sb, po, mybir.ActivationFunctionType.Tanh, bias=bias_sb
                )
                return t_sb

            loaded = {}

            def load(t):
                sb = xpool.tile([P, 2 * FW], F16, tag="sb")
                if t < NBT - 1:
                    # Two 1 MiB DMAs per tile: extra splits cost ~2% of
                    # stream rate (more engine boundaries) for no
                    # mid-stream benefit.
                    cuts = [0, FW, 2 * FW]
                else:
                    # Final tile: fine k-splits. A DMA's completion
                    # semaphore fires ~1-2 us AFTER its last byte (HBM
                    # write-receipt round trip), so chunked sems let PE
                    # start each matmul group right as data lands; the
                    # post-last-byte path is then k7's receipt + one
                    # N=512 matmul + the pipelined half-chains.
                    cuts = [0, 4 * BT, 8 * BT,
                            12 * BT, 14 * BT, 15 * BT, 16 * BT]
                for a, b in zip(cuts[:-1], cuts[1:]):
                    nc.sync.dma_start(
                        out=sb[:, a:b],
                        in_=xc[t][:, a:b],
                    )
                loaded[t] = sb

            def compute(t):
                sb = loaded.pop(t)
                # pending reduce of the previous tile rides between the
                # two matmul groups so PE never waits on tanh->mul.
                t1 = mm_group(sb, 0, mid=flush_pending)
                if t < NBT - 1:
                    t2 = mm_group(sb, 1)
                    prod = apool.tile([P, BT], F16, tag="prod")
                    nc.vector.tensor_mul(prod, t1, t2)
                    pending.append((prod, t * BT, t))
                    return
                # Final tile: run tanh/mul/reduce/sigmoid/store in two
                # 256-column half-chains so the engines pipeline instead
                # of serializing one full-width chain after the last
                # matmul (ACT does half B's tanh while DVE multiplies
                # half A, etc).
                base = FW
                po = opool.tile([P, BT], F32, name="po1", tag="po")
                for k in range(KC):
                    nc.tensor.matmul(
                        po,
                        wt_sb[:, k, :],
                        sb[:, base + k * BT:base + (k + 1) * BT],
                        start=(k == 0),
                        stop=(k == KC - 1),
                        skip_group_check=True,
                    )
                H = BT // 2
                for h in range(2):
                    cs = slice(h * H, (h + 1) * H)
                    t2h = apool.tile([P, H], F16, tag="t1")
                    nc.scalar.activation(
                        t2h, po[:, cs], mybir.ActivationFunctionType.Tanh,
                        bias=bias_sb,
                    )
                    prod = apool.tile([P, H], F16, tag="prod")
                    nc.vector.tensor_mul(prod, t1[:, cs], t2h)
                    psim = opool.tile([P, H], F32, name="psim", tag="po")
                    nc.tensor.matmul(
                        psim, ones_sb, prod,
                        start=True, stop=True, skip_group_check=True,
                    )
                    sig = apool.tile([P, H], F32, tag="sig")
                    nc.scalar.activation(
                        sig, psim, mybir.ActivationFunctionType.Sigmoid
                    )
                    row = ((2 * t + h) * 4) % P
                    nc.sync.dma_start(
                        out=out[t * BT + h * H:t * BT + (h + 1) * H]
                        .rearrange("(a n) -> a n", a=1),
                        in_=sig[row:row + 1, :],
                    )

            # Issue order: x tile 0 first on the sync ring (it IS the
            # stream bottleneck); wt/bias/ones ride the otherwise-idle
            # scalar ring (wt gates the first matmul, bias the first
            # tanh, ones the first reduce).
            load(0)
            nc.scalar.dma_start(out=wt_sb, in_=wt)
            nc.scalar.dma_start(out=bias_sb, in_=bias)
            nc.scalar.dma_start(out=ones_sb, in_=ones)
            for t in range(1, NBT):
                load(t)
                compute(t - 1)
            compute(NBT - 1)
            flush_pending()

    nc.compile()
    return nc


_NC_CACHE = None


def _get_module():
    global _NC_CACHE
    if _NC_CACHE is None:
        _NC_CACHE = _build_module()
    return _NC_CACHE


def _pack_x(x):
    """[B, D_IN] fp32 -> [N_CORES, NBT, P, FW] fp16 PE-ready tiles.

    Slot t holds tile t's transposed layout [p, k*BT + b].
    """
    xh = np.asarray(x, dtype=np.float32).astype(np.float16)
    a = xh.reshape(N_CORES, NBT, BT, KC, P).transpose(0, 1, 4, 3, 2)
    return np.ascontiguousarray(a).reshape(N_CORES, NBT, P, FW)


def _pack_inputs(x1, x2, W, b):
    f1 = _pack_x(x1)
    f2 = _pack_x(x2)
    xc_all = np.concatenate([f1, f2], axis=3)  # [c, t, p, 2*FW]
    wt = np.ascontiguousarray(
        np.asarray(W, dtype=np.float32).T.reshape(KC, P, D_PROJ)
        .transpose(1, 0, 2)
    ).astype(np.float16)
    bias = np.ascontiguousarray(np.asarray(b, dtype=np.float32).reshape(P, 1))
    ones = np.ones((P, P), dtype=np.float16)
    return [
        {
            "xc": np.ascontiguousarray(xc_all[i]),
            "wt": wt,
            "bias": bias,
            "ones": ones,
        }
        for i in range(N_CORES)
    ]


def kernel(x1, x2, W, b):
    nc = _get_module()
    in_maps = _pack_inputs(x1, x2, W, b)
    res = run_bass_kernel_spmd(nc, in_maps, core_ids=list(range(N_CORES)))
    return np.concatenate([res.results[i]["out"] for i in range(N_CORES)])


# revision 18
# speedup vs baseline: 1.1732x; 1.0020x over previous
"""Trainium2 Bass kernel: sigmoid(rowdot(tanh(x1@W.T+b), tanh(x2@W.T+b))).

Sharding: pure data-parallel over batch across 8 NeuronCores (B=65536
total -> 8192 rows/core, D_IN=1024, D_PROJ=128).

The kernel is DMA-bound on the activation loads, so the host pre-packs
x1/x2 into fp16 (end-to-end max rel err ~5e-3 vs the 2e-2 gate, measured
on the reference distribution) and into the exact PE-ready transposed
tile layout, halving HBM traffic to 32 MiB/core (~86 us at the measured
~394 GB/s per-NC DMA rate, which is the 16-SDMA-engine limit) and
eliminating every on-device PE transpose:

  xc[t][p, s*4096 + k*BT + b] = xs[t*BT + b, k*128 + p]   (s=0: x1, 1: x2)

Each 512-row tile is loaded by TWO 1 MiB contiguous DMAs (x1 half, x2
half) so the matmuls on x1 start as soon as that half's completion
semaphore fires — one fused DMA per tile would leave PE a full tile
(~5 us) behind the stream, which materializes as drain at the end. The
last tile's x2 half is further split k0-5/k6/k7 so the post-last-byte
critical path is one N=512 matmul + tanh/mul/reduce/sigmoid + one 2 KiB
store (~5 us total drain).

Per tile: 8 fp16 matmuls (N=512, 1 cyc/row warm at 2.4 GHz) accumulate
oT=W.T@xT chunkwise into one PSUM bank; ACT fuses tanh(po+bias)
PSUM->SBUF; same for x2; DVE multiplies; PE reduces partitions via
ones[128,128] matmul (f32r); ACT sigmoid; 2 KiB store from a rotating
partition. PE load is ~4.2 us/tile vs ~5.2 us/tile of DMA, so only the
partition reduce needs manual deferral (emitted between the next tile's
two matmul groups) to avoid an in-order PE stall behind the tanh->mul
chain. wt/bias/ones ride the scalar-engine DMA ring so the sync ring is
purely the x stream; output stores also use the scalar ring.
"""

import numpy as np

import concourse.bacc as bacc
import concourse.mybir as mybir
import concourse.tile as tile
from concourse.bass_utils import run_bass_kernel_spmd

N_CORES = 8
B_TOTAL = 65536
BSH = B_TOTAL // N_CORES  # 8192 rows per core
D_IN = 1024
D_PROJ = 128
P = 128
BT = 512                 # batch tile (matmul moving dim)
NBT = BSH // BT          # 16 batch tiles per core
KC = D_IN // P           # 8 contraction chunks
FW = KC * BT             # 4096 free-dim elements per packed half-tile

F32 = mybir.dt.float32
F32R = mybir.dt.float32r
F16 = mybir.dt.float16


def _build_module():
    nc = bacc.Bacc("TRN2", target_bir_lowering=False, debug=False)

    xc = nc.dram_tensor("xc", [NBT, P, 2 * FW], F16, kind="ExternalInput").ap()
    wt = nc.dram_tensor("wt", [P, KC, D_PROJ], F16, kind="ExternalInput").ap()
    bias = nc.dram_tensor("bias", [P, 1], F32, kind="ExternalInput").ap()
    ones = nc.dram_tensor("ones", [P, P], F16, kind="ExternalInput").ap()
    out = nc.dram_tensor("out", [BSH], F32, kind="ExternalOutput").ap()

    with tile.TileContext(nc) as tc:
        with (
            tc.tile_pool(name="consts", bufs=1) as cpool,
            tc.tile_pool(name="x", bufs=4) as xpool,
            tc.tile_pool(name="acts", bufs=2) as apool,
            tc.tile_pool(name="po", bufs=6, space="PSUM") as opool,
        ):
            wt_sb = cpool.tile([P, KC, D_PROJ], F16, tag="wt")
            bias_sb = cpool.tile([P, 1], F32, tag="bias")
            ones_sb = cpool.tile([P, P], F16, tag="ones")

            pending = []

            def flush_pending():
                while pending:
                    prod_p, row0_p, idx_p = pending.pop(0)
                    psim = opool.tile([P, BT], F32, name="psim", tag="po")
                    nc.tensor.matmul(
                        psim,
                        ones_sb,
                        prod_p,
                        start=True,
                        stop=True,
                        skip_group_check=True,
                    )
                    sig = apool.tile([P, BT], F32, tag="sig")
                    nc.scalar.activation(
                        sig, psim, mybir.ActivationFunctionType.Sigmoid
                    )
                    row = (idx_p * 4) % P  # rotate partition -> spread DMA engines
                    nc.scalar.dma_start(
                        out=out[row0_p:row0_p + BT].rearrange(
                            "(a n) -> a n", a=1
                        ),
                        in_=sig[row:row + 1, :],
                    )

            def mm_group(sb, tens, mid=None):
                base = tens * FW
                po = opool.tile([P, BT], F32, name=f"po{tens}", tag="po")
                for k in range(KC):
                    nc.tensor.matmul(
                        po,
                        wt_sb[:, k, :],
                        sb[:, base + k * BT:base + (k + 1) * BT],
                        start=(k == 0),
                        stop=(k == KC - 1),
                        skip_group_check=True,
                    )
                    if k == 2 and mid is not None:
                        mid()
                t_sb = apool.tile([P, BT], F16, tag=f"t{tens}")
                nc.scalar.activation(
                    t_sb, po, mybir.ActivationFunctionType.Tanh, bias=bias_sb
                )
                return t_sb

            loaded = {}

            def load(t):
                sb = xpool.tile([P, 2 * FW], F16, tag="sb")
                if t < NBT - 3:
                    # Two 1 MiB DMAs per tile: extra splits cost ~2% of
                    # stream rate (more engine boundaries) for no
                    # mid-stream benefit.
                    cuts = [0, FW, 2 * FW]
                elif t < NBT - 1:
                    # Penultimate tiles: tiny trailing x2-k7 chunk whose
                    # small-DMA receipt (~0.8 us vs ~2 us) clears the
                    # per-tile PE phase lag before the final window.
                    cuts = [0, FW, 15 * BT, 16 * BT]
                else:
                    # Final tile: fine k-splits. A DMA's completion
                    # semaphore fires ~1-2 us AFTER its last byte (HBM
                    # write-receipt round trip), so chunked sems let PE
                    # start each matmul group right as data lands; the
                    # post-last-byte path is then k7's receipt + one
                    # N=512 matmul + the pipelined half-chains.
                    cuts = [0, 4 * BT, 8 * BT,
                            12 * BT, 14 * BT, 15 * BT, 16 * BT]
                for a, b in zip(cuts[:-1], cuts[1:]):
                    nc.sync.dma_start(
                        out=sb[:, a:b],
                        in_=xc[t][:, a:b],
                    )
                loaded[t] = sb

            def compute(t):
                sb = loaded.pop(t)
                # pending reduce of the previous tile rides between the
                # two matmul groups so PE never waits on tanh->mul.
                t1 = mm_group(sb, 0, mid=flush_pending)
                if t < NBT - 1:
                    t2 = mm_group(sb, 1)
                    prod = apool.tile([P, BT], F16, tag="prod")
                    nc.vector.tensor_mul(prod, t1, t2)
                    pending.append((prod, t * BT, t))
                    return
                # Final tile: run tanh/mul/reduce/sigmoid/store in two
                # 256-column half-chains so the engines pipeline instead
                # of serializing one full-width chain after the last
                # matmul (ACT does half B's tanh while DVE multiplies
                # half A, etc).
                base = FW
                po = opool.tile([P, BT], F32, name="po1", tag="po")
                for k in range(KC):
                    nc.tensor.matmul(
                        po,
                        wt_sb[:, k, :],
                        sb[:, base + k * BT:base + (k + 1) * BT],
                        start=(k == 0),
                        stop=(k == KC - 1),
                        skip_group_check=True,
                    )
                H = BT // 2
                for h in range(2):
                    cs = slice(h * H, (h + 1) * H)
                    t2h = apool.tile([P, H], F16, tag="t1")
                    nc.scalar.activation(
                        t2h, po[:, cs], mybir.ActivationFunctionType.Tanh,
                        bias=bias_sb,
                    )
                    prod = apool.tile([P, H], F16, tag="prod")
                    nc.vector.tensor_mul(prod, t1[:, cs], t2h)
                    psim = opool.tile([P, H], F32, name="psim", tag="po")
                    nc.tensor.matmul(
                        psim, ones_sb, prod,
                        start=True, stop=True, skip_group_check=True,
                    )
                    sig = apool.tile([P, H], F32, tag="sig")
                    nc.scalar.activation(
                        sig, psim, mybir.ActivationFunctionType.Sigmoid
                    )
                    row = ((2 * t + h) * 4) % P
                    # h0 via the (idle) sync ring; h1 via the scalar
                    # ring right behind its own sigmoid so neither store
                    # issue blocks the other chain's ACT ops.
                    eng = nc.sync if h == 0 else nc.scalar
                    eng.dma_start(
                        out=out[t * BT + h * H:t * BT + (h + 1) * H]
                        .rearrange("(a n) -> a n", a=1),
                        in_=sig[row:row + 1, :],
                    )

            # Issue order: x tile 0 first on the sync ring (it IS the
            # stream bottleneck); wt/bias/ones ride the otherwise-idle
            # scalar ring (wt gates the first matmul, bias the first
            # tanh, ones the first reduce).
            load(0)
            nc.scalar.dma_start(out=wt_sb, in_=wt)
            nc.scalar.dma_start(out=bias_sb, in_=bias)
            nc.scalar.dma_start(out=ones_sb, in_=ones)
            for t in range(1, NBT):
                load(t)
                compute(t - 1)
            compute(NBT - 1)
            flush_pending()

    nc.compile()
    return nc


_NC_CACHE = None


def _get_module():
    global _NC_CACHE
    if _NC_CACHE is None:
        _NC_CACHE = _build_module()
    return _NC_CACHE


def _pack_x(x):
    """[B, D_IN] fp32 -> [N_CORES, NBT, P, FW] fp16 PE-ready tiles.

    Slot t holds tile t's transposed layout [p, k*BT + b].
    """
    xh = np.asarray(x, dtype=np.float32).astype(np.float16)
    a = xh.reshape(N_CORES, NBT, BT, KC, P).transpose(0, 1, 4, 3, 2)
    return np.ascontiguousarray(a).reshape(N_CORES, NBT, P, FW)


def _pack_inputs(x1, x2, W, b):
    f1 = _pack_x(x1)
    f2 = _pack_x(x2)
    xc_all = np.concatenate([f1, f2], axis=3)  # [c, t, p, 2*FW]
    wt = np.ascontiguousarray(
        np.asarray(W, dtype=np.float32).T.reshape(KC, P, D_PROJ)
        .transpose(1, 0, 2)
    ).astype(np.float16)
    bias = np.ascontiguousarray(np.asarray(b, dtype=np.float32).reshape(P, 1))
    ones = np.ones((P, P), dtype=np.float16)
    return [
        {
            "xc": np.ascontiguousarray(xc_all[i]),
            "wt": wt,
            "bias": bias,
            "ones": ones,
        }
        for i in range(N_CORES)
    ]


def kernel(x1, x2, W, b):
    nc = _get_module()
    in_maps = _pack_inputs(x1, x2, W, b)
    res = run_bass_kernel_spmd(nc, in_maps, core_ids=list(range(N_CORES)))
    return np.concatenate([res.results[i]["out"] for i in range(N_CORES)])
